# revision 15
# baseline (speedup 1.0000x reference)
"""Trainium2 Bass kernel: sqrt-scaled affinity + relu-pow(2/3) + causally masked cumsum.

Per (b,h) pair:
    s  = sqrt(src + ERR); d = sqrt(dest + ERR)
    a1 = (k * s[:, None]) @ (k * d[:, None]).T          # [L, L]
    a3 = (relu(a1) + ERR) ** (2/3)                      # exp((2/3) * ln(.))
    out = where(j >= i, cumsum_j(a3), 0)

Sharding: 32 (b,h) pairs -> 4 per NeuronCore x 8 cores, fully independent.

Device mapping per pair (s/d scaling is factored out of the matmul:
relu(s_i d_j m) = s_i * d_j * relu(m) since s,d > 0):
    - PE computes m = K K^T from a single transposed copy kT [D, L] (f32r for
      the 1-cycle/row PE path), row-strip [128, L] at a time into PSUM
    - DVE scalar_tensor_tensor: r = max(m, 0) * d_row  (PSUM -> SBUF), where
      d_row is a [128, L] broadcast of d built once per pair via DMA
    - ACT: t = ln(s_i * r + ERR) (per-partition scale AP + bias AP),
      e = exp((2/3) t); the left-of-diagonal region also emits accum_out =
      per-row sum (the cumsum carry) so it needs no scan
    - DVE tensor_tensor_scan (add/bypass) prefix-sums the right region,
      seeded with the carry
    - GPSIMD affine_select zeroes the below-diagonal triangle of the one
      diagonal 128x128 sub-block
    - Only columns >= the diagonal are DMA'd out; the runtime pre-zeroes the
      output buffer so the lower triangle stays 0.
"""

import numpy as np

B, H, L, D = 2, 16, 2048, 64
N_CORES = 8
Q = (B * H) // N_CORES  # pairs per core = 4
P = 128                 # partitions / i-tile height
JBLK = 512              # fp32 PSUM bank width / j-block
ERR = 1e-12
TWO_THIRDS = 2.0 / 3.0


def build_program(n_pairs=Q, l=L, d=D):
    import concourse.bacc as bacc
    import concourse.mybir as mybir
    from concourse.tile import TileContext
    from concourse.masks import make_identity

    f32 = mybir.dt.float32
    f32r = mybir.dt.float32r
    Alu = mybir.AluOpType
    Act = mybir.ActivationFunctionType

    ti_n = l // P
    assert l % (2 * JBLK) == 0 and ti_n % 4 == 0 and d <= 128

    nc = bacc.Bacc(None, target_bir_lowering=False)
    k_in = nc.declare_dram_parameter("k", [n_pairs, l, d], f32, isOutput=False)
    src_in = nc.declare_dram_parameter("src", [n_pairs, l], f32, isOutput=False)
    dest_in = nc.declare_dram_parameter("dest", [n_pairs, l], f32, isOutput=False)
    out_t = nc.declare_dram_parameter("out", [n_pairs, l, l], f32, isOutput=True)

    with TileContext(nc) as tc:
        with (
            tc.tile_pool(name="singles", bufs=1) as singles,
            tc.tile_pool(name="kraw", bufs=2) as kraw_pool,
            tc.tile_pool(name="kt", bufs=2) as kt_pool,
            tc.tile_pool(name="drow", bufs=2) as drow_pool,
            tc.tile_pool(name="work", bufs=3) as work_pool,
            tc.tile_pool(name="carry", bufs=4) as carry_pool,
            tc.tile_pool(name="pa1", bufs=3, space="PSUM") as pa1_pool,
            tc.tile_pool(name="ptp", bufs=1, space="PSUM") as ptp_pool,
            tc.tile_pool(name="dram", bufs=1, space="DRAM") as dram_pool,
        ):
            # DRAM bounce for building the d-row broadcast (partition crossing)
            d_bounce = dram_pool.tile([n_pairs * ti_n, P], f32)
            identity = singles.tile([P, P], f32)
            make_identity(nc, identity)

            err_col = singles.tile([P, 1], f32)
            nc.vector.memset(err_col, ERR)

            # sqrt-scaled source/dest weights for all pairs upfront so the
            # sqrt activation table is loaded exactly once.
            sall = singles.tile([P, n_pairs * ti_n], f32)
            dall = singles.tile([P, n_pairs * ti_n], f32)
            for q in range(n_pairs):
                nc.sync.dma_start(
                    out=sall[:, q * ti_n:(q + 1) * ti_n],
                    in_=src_in[q].rearrange("(t p) -> p t", p=P),
                )
                nc.sync.dma_start(
                    out=dall[:, q * ti_n:(q + 1) * ti_n],
                    in_=dest_in[q].rearrange("(t p) -> p t", p=P),
                )
            nc.scalar.activation(sall, sall, Act.Sqrt, bias=err_col)
            nc.scalar.activation(dall, dall, Act.Sqrt, bias=err_col)

            # d in row layout: transpose dall -> DRAM bounce; each pair later
            # DMA-broadcasts its row across all 128 partitions.
            ps_dall = ptp_pool.tile([n_pairs * ti_n, P], f32, tag="tp")
            nc.tensor.transpose(ps_dall, dall, identity)
            dallT = singles.tile([n_pairs * ti_n, P], f32)
            nc.vector.tensor_copy(out=dallT, in_=ps_dall)
            nc.sync.dma_start(out=d_bounce[:, :], in_=dallT)

            kts = {}

            def emit_prep(q):
                # load k, transpose via PE to kT [d, l] (f32r), and build the
                # [128, l] broadcast of this pair's d row
                kraw = kraw_pool.tile([P, ti_n, d], f32, tag="kraw")
                nc.sync.dma_start(
                    out=kraw, in_=k_in[q].rearrange("(t p) d -> p t d", p=P)
                )
                kT = kt_pool.tile([d, l], f32r, tag="kT")
                for g in range(ti_n // 4):
                    ps = ptp_pool.tile([d, 4 * P], f32, tag="tp")
                    for u in range(4):
                        t = 4 * g + u
                        nc.tensor.transpose(ps[:, u * P:(u + 1) * P], kraw[:, t], identity)
                    nc.scalar.copy(out=kT[:, g * 4 * P:(g + 1) * 4 * P], in_=ps)
                d_row = drow_pool.tile([P, l], f32, tag="d_row")
                src_ap = d_bounce[q * ti_n:(q + 1) * ti_n, :].rearrange(
                    "t p -> (t p)"
                ).partition_broadcast(P)
                nc.sync.dma_start(out=d_row, in_=src_ap)
                kts[q] = (kT, d_row)

            def emit_scan_out(pend):
                # prefix-sum the right region, mask the diagonal sub-block,
                # DMA the masked row-strip out
                e, init, c0, q = pend
                cs = work_pool.tile([P, l], f32, tag="cs")
                nc.vector.tensor_tensor_scan(
                    cs[:, c0:], e[:, c0:], e[:, c0:], init,
                    op0=Alu.add, op1=Alu.bypass,
                )
                nc.gpsimd.affine_select(
                    out=cs[:, c0:c0 + P],
                    in_=cs[:, c0:c0 + P],
                    pattern=[[1, P]],
                    compare_op=Alu.is_ge,
                    fill=0.0,
                    base=0,
                    channel_multiplier=-1,
                )
                nc.sync.dma_start(
                    out=out_t[q, c0:c0 + P, c0:l], in_=cs[:, c0:l]
                )

            emit_prep(0)
            pending = None  # software pipeline: scan(ti) emitted after relu(ti+1)
            w = l // 2
            for q in range(n_pairs):
                kT, d_row = kts[q]
                for ti in range(ti_n):
                    if ti == 2 and q + 1 < n_pairs:
                        emit_prep(q + 1)
                    c0 = ti * P  # diagonal / first output column
                    s_col = sall[:, q * ti_n + ti: q * ti_n + ti + 1]
                    r = work_pool.tile([P, l], f32, tag="r")
                    for hh in range(2):
                        a1 = pa1_pool.tile([P, w], f32, tag="a1")
                        for jb in range(w // JBLK):
                            nc.tensor.matmul(
                                a1[:, jb * JBLK:(jb + 1) * JBLK],
                                lhsT=kT[:, c0:c0 + P],
                                rhs=kT[:, hh * w + jb * JBLK: hh * w + (jb + 1) * JBLK],
                                start=True,
                                stop=True,
                            )
                        # r = max(m, 0) * d_j  (PSUM -> SBUF)
                        nc.vector.scalar_tensor_tensor(
                            out=r[:, hh * w:(hh + 1) * w],
                            in0=a1,
                            scalar=0.0,
                            in1=d_row[:, hh * w:(hh + 1) * w],
                            op0=Alu.max,
                            op1=Alu.mult,
                        )
                    # t = ln(s_i * r + ERR)
                    t_ = work_pool.tile([P, l], f32, tag="t")
                    nc.scalar.activation(t_, r, Act.Ln, bias=err_col, scale=s_col)
                    e = work_pool.tile([P, l], f32, tag="e")
                    if c0 > 0:
                        carry = carry_pool.tile([P, 1], f32, tag="carry")
                        nc.scalar.activation(
                            e[:, :c0], t_[:, :c0], Act.Exp,
                            scale=TWO_THIRDS, accum_out=carry,
                        )
                        init = carry
                    else:
                        init = 0.0
                    nc.scalar.activation(
                        e[:, c0:], t_[:, c0:], Act.Exp, scale=TWO_THIRDS
                    )
                    if pending is not None:
                        emit_scan_out(pending)
                    pending = (e, init, c0, q)
            emit_scan_out(pending)

    # The act-table chooser picks a set per-activation greedily; Ln and Exp
    # land in different sets and thrash a ~2.7us table load per i-tile. Hide
    # Ln/Exp from every set except the combined one so both use it (it
    # genuinely contains both functions, so the emitted program is HW-valid).
    import concourse.bacc as bacc_mod

    orig_tables = bacc_mod.get_activation_tables

    def _steered_tables(arch):
        tables = orig_tables(arch)
        combo = "natural_log_exp_and_others"
        if combo in tables:
            for name, funcs in tables.items():
                if name != combo:
                    funcs.discard(Act.Ln)
                    funcs.discard(Act.Exp)
        return tables

    bacc_mod.get_activation_tables = _steered_tables
    try:
        nc.compile()
    finally:
        bacc_mod.get_activation_tables = orig_tables
    return nc


_PROG = None


def _get_program():
    global _PROG
    if _PROG is None:
        _PROG = build_program()
    return _PROG


def kernel(k, src, dest):
    from concourse.bass_utils import run_bass_kernel_spmd

    nc = _get_program()
    kf = np.ascontiguousarray(np.asarray(k, dtype=np.float32).reshape(B * H, L, D))
    sf = np.ascontiguousarray(np.asarray(src, dtype=np.float32).reshape(B * H, L))
    df = np.ascontiguousarray(np.asarray(dest, dtype=np.float32).reshape(B * H, L))
    in_maps = [
        {"k": kf[c * Q:(c + 1) * Q], "src": sf[c * Q:(c + 1) * Q], "dest": df[c * Q:(c + 1) * Q]}
        for c in range(N_CORES)
    ]
    res = run_bass_kernel_spmd(nc, in_maps, list(range(N_CORES)))
    out = np.concatenate([res.results[c]["out"] for c in range(N_CORES)], axis=0)
    return out.reshape(B, H, L, L)
